# revision 1
# baseline (speedup 1.0000x reference)
"""Bass/Trainium2 kernel for nn_ADJ_FirstLayer (gnn_message_passing).

reference(x):  N = x.shape[0]; M = N + 4
  A = eye(M); A[N:, N:] = 1  (symmetric)
  d = rowsum(A)^-0.5  ->  d[i] = 1 for i < N, 0.5 for i >= N
  out = d[:,None] * A.T * d[None,:]
  => out = identity on first N diagonal entries, bottom-right 4x4 block = 0.25

The output depends only on N, not on x's values, and is 99.99% zeros:
a pure HBM-write-bandwidth problem (M*M*4 bytes = 268.7 MB).

Sharding: row-shard the (M x M) output across 8 cores, R = 1025 rows each
(8*1025 = 8200 >= 8196; the host trims the last 4 garbage rows). Each core
zero-fills its (R, M) block with large SBUF->DRAM DMAs, then writes its
piece of the diagonal with a dynamic-offset strided DMA (stride M+1). All
rank-dependence (diagonal column offset, diagonal values, 4x4 corner
values) is carried in tiny per-core input arrays; the SPMD program is
identical on every core. x itself never touches the device - it does not
appear in the math.

Measured-on-HW design notes (fast-mode HW exec ~96 us; contention mode
~110 us; do-nothing-NEFF floor ~12.7 us):
 - The NEFF preamble (EVSEM butterfly + tensor loads) keeps every engine
   busy until ~7.6 us; nothing user-issued can start earlier. Sync (SP)
   issues first; Vector/GpSimd wake ~7.6-8.9 us.
 - The zero body is streamed as A + B. A (512 KB) is DRAM->DRAM from a
   host-zero input: no SBUF dependency, so it streams from ~8.3 us while
   the zero-tile memsets (split asymmetrically across GpSimd and Vector)
   finish; B (33 MB) is ONE broadcast-source DMA (the [128, 4032] zeroed
   SBUF span re-read 16x via a stride-0 middle dim, ~15.75 KB
   descriptors).
 - Descriptor size matters (2 KB descs cost ~12%, ~11 ns/descriptor),
   but the residual ~5 us wall-vs-busy gap inside the stream is
   invariant to descriptor size AND to dst traversal order (bank-spread
   tested) - it is HBM-side pacing, not addressable from the kernel.
 - Splitting the stream across two HWDGE rings (sync+scalar) REGRESSED
   ~17% (engine round-robin stalls); only tiny DMAs (remainder, tail,
   corner block) go on the scalar ring, in parallel with the sync ring's
   main-diagonal write.
 - The 1024-element main diagonal is written as TWO 512-cell halves at
   two dynamic offsets, one per HWDGE ring (sync + scalar), each half's
   values packed across all 128 SBUF partitions so its descriptors fan
   over all 16 SDMA engines. Versus one 1024-descriptor DMA this is
   ~2 us faster (parallel descriptor generation + processing). A
   [1, 1024] source would pin every descriptor to engine 0 (~11 us
   serial tail).
 - Row 1024's diagonal element is written separately (1-element DMA at a
   third dynamic offset); on core 7 that write is aimed at a trimmed
   garbage row.
"""
import sys

if "/opt/trn_rl_repo" not in sys.path:
    sys.path.insert(0, "/opt/trn_rl_repo")

import numpy as np

import concourse.bass as bass
from concourse import mybir
from concourse.bass_utils import run_bass_kernel_spmd


def _ensure_axon_hooks():
    """bass_utils' trace path does `from antenv.axon_hooks import ...`
    unconditionally; this image's antenv lacks that module, which would
    crash any BASS_TRACE=1 run. Inject it (with the ctypes NTFF hook when
    available) so tracing works instead of raising."""
    import types

    if "antenv.axon_hooks" in sys.modules:
        return
    hook = None
    try:
        if "/root/.axon_site" not in sys.path:
            sys.path.insert(0, "/root/.axon_site")
        from trn_agent_boot.trn_boot import _ntff_profile_via_ctypes

        hook = _ntff_profile_via_ctypes("/opt/axon/libaxon_pjrt.so")
    except Exception:
        hook = None
    mod = types.ModuleType("antenv.axon_hooks")
    mod._hook = hook
    mod.get_axon_ntff_profile_hook = lambda: mod._hook
    mod.set_axon_ntff_profile_hook = lambda h: setattr(mod, "_hook", h)
    sys.modules["antenv.axon_hooks"] = mod


_ensure_axon_hooks()

N = 8192
M = N + 4            # 8196
N_CORES = 8
R = 1025             # rows per core; 8*1025 = 8200, host trims to 8196
FLAT = R * M         # 8,400,900 elements per core

DIAG_MAIN = 1024     # main diagonal segment length = 128 partitions * 8
BLK_ROW0 = 1017      # local row of the 4x4 ones block on core 7

ZT = 4096            # zero-tile SBUF extent [128, ZT] f32
BIG = 128 * 16 * ZT               # 8,388,608 elements (32 MB) zero body
REM = FLAT - BIG                  # 12,292 = 4 * 3073
REM_P, REM_F = 4, 3073
# The zero body is written as A + B:
#  A: 512 KB DRAM->DRAM from a host-zero input - needs no SBUF, so it
#     launches at engine wake (~8 us) and streams while the memsets run.
#  B: the rest, broadcast-sourced from the SBUF zero tile (span 4032,
#     15.75 KB descriptors), issued as soon as the memsets finish.
A_ELEMS = 128 * 1024              # 131,072 elements (512 KB)
B_SPAN = 3900                     # B-main source span (15.2 KB descs)
B_REP = 16                        # B-main = 128 * 16 * 3900 = 7,987,200
HB_OFF = A_ELEMS + 128 * B_REP * B_SPAN   # 8,118,272
HB_ELEMS = FLAT - HB_OFF                  # 282,628 baked tail elements
HB1 = 64 * 4096                           # baked tail part 1 (16 KB spans)
HB2 = HB_ELEMS - HB1                      # 20,484-element flat remainder
MS_G = 2304          # gpsimd memsets [0:MS_G), vector [MS_G:B_SPAN)
                     # (gpsimd wakes ~0.9us earlier; finish ~10.1/10.4 us)

C1_MAX = (N_CORES - 2) * R + DIAG_MAIN * (M + 1)  # largest tail offset (core 6)

_nc_cache = None


def _build():
    nc = bass.Bass()
    zsrc = nc.declare_dram_parameter("zsrc", [128, 1024], mybir.dt.float32, isOutput=False)
    dvals = nc.declare_dram_parameter("dvals", [128, 8], mybir.dt.float32, isOutput=False)
    hbuf1 = nc.declare_dram_parameter("hbuf1", [64, 4096], mybir.dt.float32, isOutput=False)
    hbuf2 = nc.declare_dram_parameter("hbuf2", [1, HB2], mybir.dt.float32, isOutput=False)
    offs = nc.declare_dram_parameter("offs", [1, 4], mybir.dt.int32, isOutput=False)
    out = nc.declare_dram_parameter("out", [R, M], mybir.dt.float32, isOutput=True)
    out_flat = out[:].flatten()

    with (
        nc.Block() as block,
        nc.semaphore("prep_sem") as prep_sem,
        nc.semaphore("in_sem") as in_sem,
        nc.semaphore("zdma_sem") as zdma_sem,
        nc.semaphore("hsem") as hsem,
        nc.semaphore("fdma_sem") as fdma_sem,
        nc.sbuf_tensor("ztile", [128, ZT], mybir.dt.float32) as ztile,
        nc.sbuf_tensor("dtile", [128, 8], mybir.dt.float32) as dtile,
        nc.sbuf_tensor("otile", [1, 4], mybir.dt.int32) as otile,
        nc.sync.register() as r0,
        nc.scalar.register() as r2,
    ):
        # asymmetric split: gpsimd wakes ~0.9us before vector (measured),
        # so give it a head-start-sized share despite its slower rate
        @block.vector
        def _(vector):
            vector.memset(ztile[:, MS_G:B_SPAN], 0.0).then_inc(prep_sem, 1)

        @block.gpsimd
        def _(gpsimd):
            gpsimd.memset(ztile[:, 0:MS_G], 0.0).then_inc(prep_sem, 1)

        @block.scalar
        def _(scalar):
            # diagonal cells [512, 1024) in parallel with the sync ring's
            # [0, 512); overlaps with the baked tail double-write identical
            # values (harmless)
            scalar.wait_ge(in_sem, 32)
            scalar.reg_load(r2, otile[0:1, 2:3])
            c0b = scalar.snap(r2)
            Hh = DIAG_MAIN // 2
            d0b = out_flat[0 : (N_CORES - 1) * R + Hh * (M + 1) + 1][
                bass.ds(c0b, 1)
            ].offset
            mainb_ap = bass.AP(out_flat.tensor, d0b, [[M + 1, Hh]])
            scalar.wait_ge(zdma_sem, 32)
            with nc.allow_non_contiguous_dma(reason="diagonal scatter"):
                scalar.dma_start(out=mainb_ap, in_=dtile[:, 4:8]).then_inc(fdma_sem, 16)

        @block.sync
        def _(sync):
            # A: DRAM->DRAM zeros, no SBUF dependency - streams immediately
            dst = bass.AP(out_flat.tensor, 0, [[1, A_ELEMS]])
            sync.dma_start(out=dst, in_=zsrc[:, :]).then_inc(zdma_sem, 16)
            sync.dma_start(out=dtile[:, :], in_=dvals[:, :]).then_inc(in_sem, 16)
            sync.dma_start(out=otile[:, :], in_=offs[:, :]).then_inc(in_sem, 16)
            sync.wait_ge(prep_sem, 2)
            zap = ztile[:, :]
            # B: the remaining 31.5 MB, broadcast-sourced; bank-spread
            # traversal (consecutive descriptors of one engine jump ~2 MB)
            dst = bass.AP(out_flat.tensor, A_ELEMS,
                          [[B_SPAN, 128], [128 * B_SPAN, B_REP], [1, B_SPAN]])
            src = bass.AP(zap.tensor, zap.offset,
                          [[zap.ap[0][0], 128], [0, B_REP], [1, B_SPAN]])
            sync.dma_start(out=dst, in_=src).then_inc(zdma_sem, 16)
            # baked tail: zeros + all late values (late diag cells, corner
            # block, row-1024 element, trim rows), DRAM->DRAM from host
            # buffers; geometry clones the proven zsrc/A patterns
            dst = bass.AP(out_flat.tensor, HB_OFF, [[4096, 64], [1, 4096]])
            sync.dma_start(out=dst, in_=hbuf1[:, :]).then_inc(hsem, 16)
            dst = bass.AP(out_flat.tensor, HB_OFF + HB1, [[1, HB2]])
            sync.dma_start(out=dst, in_=hbuf2[:, :]).then_inc(hsem, 16)

            # load diagonal offset while the zero stream runs
            sync.wait_ge(in_sem, 32)
            sync.reg_load(r0, otile[0:1, 0:1])
            c0 = sync.snap(r0)
            d0 = out_flat[0 : (N_CORES - 1) * R + 1][bass.ds(c0, 1)].offset
            main_ap = bass.AP(out_flat.tensor, d0, [[M + 1, DIAG_MAIN // 2]])

            sync.wait_ge(zdma_sem, 32)
            # first half of the main diagonal (stride M+1); cells spread
            # across all 128 source partitions so descriptors hit all 16
            # engines; second half runs concurrently on the scalar ring
            with nc.allow_non_contiguous_dma(reason="diagonal scatter"):
                sync.dma_start(out=main_ap, in_=dtile[:, 0:4]).then_inc(fdma_sem, 16)
            sync.wait_ge(fdma_sem, 32)
            sync.wait_ge(hsem, 32)
    return nc


def _in_maps():
    maps = []
    zsrc = np.zeros((128, 1024), np.float32)
    H = DIAG_MAIN // 2
    for r in range(N_CORES):
        vals = np.ones(DIAG_MAIN, np.float32)  # diagonal values, in diag order
        svals = np.zeros((1, 24), np.float32)
        offs = np.zeros((1, 4), np.int32)
        c0 = r * R
        if r < N_CORES - 1:
            svals[0, 0] = 1.0                  # row-1024 diagonal element
            c1 = c0 + DIAG_MAIN * (M + 1)
        else:
            # core 7: global rows 7175..8199; 8192..8195 hold the ones-block,
            # 8196..8199 are trimmed garbage.
            vals[BLK_ROW0 : BLK_ROW0 + 4] = 0.25   # diag entries in the 4x4 block
            vals[BLK_ROW0 + 4 :] = 0.0             # rows 8196+: garbage, any value
            svals[0, 0] = 0.0
            svals[0, 4:20] = 0.25              # the 4x4 ones block * 0.25
            c1 = (DIAG_MAIN - 3) * M           # inside garbage row 1021
        # pack halves so each spans all 128 partitions: half-a cell k lives
        # at [k//4, k%4], half-b cell 512+k at [k//4, 4+k%4]
        dvals = np.zeros((128, 8), np.float32)
        dvals[:, 0:4] = vals[:H].reshape(128, 4)
        dvals[:, 4:8] = vals[H:].reshape(128, 4)
        offs[0, 0] = c0
        offs[0, 1] = c1
        offs[0, 2] = c0 + H * (M + 1)
        hb = np.zeros(HB_ELEMS, np.float32)
        for i in range(DIAG_MAIN):
            q = i * (M + 1) + c0
            if q >= HB_OFF:
                hb[q - HB_OFF] = vals[i]
        if r < N_CORES - 1:
            hb[c0 + DIAG_MAIN * (M + 1) - HB_OFF] = 1.0   # row-1024 diag
        else:
            for rr in range(BLK_ROW0, BLK_ROW0 + 4):      # 4x4 ones block
                hb[rr * M + N - HB_OFF : rr * M + N + 4 - HB_OFF] = 0.25
        maps.append({"zsrc": zsrc, "dvals": dvals, "svals": svals, "offs": offs,
                     "hbuf1": hb[:HB1].reshape(64, 4096),
                     "hbuf2": hb[HB1:].reshape(1, HB2)})
    return maps


def _run(trace=False, **kwargs):
    global _nc_cache
    if _nc_cache is None:
        _nc_cache = _build()
    return run_bass_kernel_spmd(
        _nc_cache, _in_maps(), core_ids=list(range(N_CORES)), trace=trace, **kwargs
    )


def kernel(x: np.ndarray) -> np.ndarray:
    assert x.shape == (N, 2048), x.shape
    res = _run()
    blocks = [res.results[r]["out"] for r in range(N_CORES)]
    return np.concatenate(blocks, axis=0)[:M]


if __name__ == "__main__":
    out = kernel(np.zeros((N, 2048), np.float32))
    print(out.shape, out.dtype)



# revision 2
# speedup vs baseline: 7.0361x; 7.0361x over previous
"""Bass/Trainium2 kernel for nn_ADJ_FirstLayer (gnn_message_passing).

reference(x):  N = x.shape[0]; M = N + 4
  A = eye(M); A[N:, N:] = 1  (symmetric)
  d = rowsum(A)^-0.5  ->  d[i] = 1 for i < N, 0.5 for i >= N
  out = d[:,None] * A.T * d[None,:]
  => out = identity on first N diagonal entries, bottom-right 4x4 block = 0.25

The output depends only on N, not on x's values, and is 99.99% zeros.

Key fact (see concourse/bass2jax.py run_bass_via_pjrt): ExternalOutput
buffers are pre-zeroed by the runtime on BOTH execution paths — the
native path memsets them before run_neff, and the axon/PJRT path
donates freshly-zeroed host buffers that XLA aliases to the NEFF output
("kernels that don't write every element rely on that"). Verified on
this hardware with a dirty-memory probe: a kernel that writes 16 cells
reads back exact zeros everywhere else, even immediately after another
kernel filled the same-sized output with garbage.

So the kernel writes ONLY the nonzero cells (~33 KB total instead of
268.7 MB): per core a 1025-cell diagonal run at stride M+1, plus the
4x4 corner block. HW exec time drops from ~110 us (full zero-fill at
the HBM write roofline) to near the bare-NEFF floor.

Sharding: row-shard the (M x M) output across 8 cores, R = 1025 rows
each (8*1025 = 8200 >= 8196; the host trims the last 4 garbage rows).
SPMD: one program, per-core behavior carried entirely by tiny input
arrays (diagonal values, dynamic offsets, corner-block values):
 - diagonal: core r's local diag cell i sits at flat i*(M+1) + r*R.
   The r*R column offset comes from an int32 input via reg_load +
   dynamic-offset AP. Cells 0..511 go on the sync ring, 512..1023 on
   the scalar ring (parallel descriptor generation), cell 1024 as a
   1-cell DMA. Values are packed across all 128 SBUF partitions so
   each half's 512 descriptors fan over all 16 SDMA engines.
 - corner block: global rows/cols [N, N+4) fall on core 7 at local
   rows 1017..1020, cols N..N+3 — a STATIC local position, so every
   core issues the same 4x4 DMA; cores 0-6 write input-supplied 0.0
   (idempotent over the pre-zeroed buffer), core 7 writes 0.25. The
   block's diagonal cells are also covered by the diag DMAs with the
   same value 0.25 — double-write of identical bytes, harmless.
 - core 7's "cell 1024" (global row 8199, trimmed) is aimed at a
   garbage row inside its own block with value 0.0.
"""
import sys

if "/opt/trn_rl_repo" not in sys.path:
    sys.path.insert(0, "/opt/trn_rl_repo")

import numpy as np

import concourse.bass as bass
from concourse import mybir
from concourse.bass_utils import run_bass_kernel_spmd


def _ensure_axon_hooks():
    """bass_utils' trace path does `from antenv.axon_hooks import ...`
    unconditionally; this image's antenv lacks that module, which would
    crash any BASS_TRACE=1 run. Inject it (with the ctypes NTFF hook when
    available) so tracing works instead of raising."""
    import types

    if "antenv.axon_hooks" in sys.modules:
        return
    hook = None
    try:
        if "/root/.axon_site" not in sys.path:
            sys.path.insert(0, "/root/.axon_site")
        from trn_agent_boot.trn_boot import _ntff_profile_via_ctypes

        hook = _ntff_profile_via_ctypes("/opt/axon/libaxon_pjrt.so")
    except Exception:
        hook = None
    mod = types.ModuleType("antenv.axon_hooks")
    mod._hook = hook
    mod.get_axon_ntff_profile_hook = lambda: mod._hook
    mod.set_axon_ntff_profile_hook = lambda h: setattr(mod, "_hook", h)
    sys.modules["antenv.axon_hooks"] = mod


_ensure_axon_hooks()

N = 8192
M = N + 4            # 8196
N_CORES = 8
R = 1025             # rows per core; 8*1025 = 8200, host trims to 8196
FLAT = R * M         # 8,400,900 elements per core

H = 512              # diagonal half length (1025 = 512 + 512 + 1)
BLK_R0 = 1017        # local row of the 4x4 ones block on core 7

C0_MAX = (N_CORES - 1) * R                   # max diag-start offset (core 7)
C1_MAX = (N_CORES - 2) * R + 2 * H * (M + 1) # max cell-1024 offset (core 6)
C2_MAX = (N_CORES - 1) * R + H * (M + 1)     # max half-B start offset (core 7)

_nc_cache = None


def _build():
    nc = bass.Bass()
    dvals = nc.declare_dram_parameter("dvals", [128, 12], mybir.dt.float32, isOutput=False)
    offs = nc.declare_dram_parameter("offs", [1, 4], mybir.dt.int32, isOutput=False)
    out = nc.declare_dram_parameter("out", [R, M], mybir.dt.float32, isOutput=True)
    out_flat = out[:].flatten()

    with (
        nc.Block() as block,
        nc.semaphore("in_sem") as in_sem,
        nc.semaphore("fdma_sem") as fdma_sem,
        nc.sbuf_tensor("dtile", [128, 12], mybir.dt.float32) as dtile,
        nc.sbuf_tensor("otile", [1, 4], mybir.dt.int32) as otile,
        nc.sync.register() as r0,
        nc.sync.register() as r1,
        nc.scalar.register() as r2,
    ):
        @block.scalar
        def _(scalar):
            # diagonal cells [512, 1024) at dynamic offset c2 = r*R + 512*(M+1)
            scalar.wait_ge(in_sem, 32)
            scalar.reg_load(r2, otile[0:1, 2:3])
            c2 = scalar.snap(r2)
            d2 = out_flat[0 : C2_MAX + 1][bass.ds(c2, 1)].offset
            apB = bass.AP(out_flat.tensor, d2, [[M + 1, H]])
            with nc.allow_non_contiguous_dma(reason="diagonal scatter"):
                scalar.dma_start(out=apB, in_=dtile[:, 4:8]).then_inc(fdma_sem, 16)

        @block.sync
        def _(sync):
            sync.dma_start(out=dtile[:, :], in_=dvals[:, :]).then_inc(in_sem, 16)
            sync.dma_start(out=otile[:, :], in_=offs[:, :]).then_inc(in_sem, 16)
            sync.wait_ge(in_sem, 32)
            # diagonal cells [0, 512) at dynamic offset c0 = r*R
            sync.reg_load(r0, otile[0:1, 0:1])
            c0 = sync.snap(r0)
            d0 = out_flat[0 : C0_MAX + 1][bass.ds(c0, 1)].offset
            apA = bass.AP(out_flat.tensor, d0, [[M + 1, H]])
            with nc.allow_non_contiguous_dma(reason="diagonal scatter"):
                sync.dma_start(out=apA, in_=dtile[:, 0:4]).then_inc(fdma_sem, 16)
            # diagonal cell 1024 at dynamic offset c1
            sync.reg_load(r1, otile[0:1, 1:2])
            c1 = sync.snap(r1)
            d1 = out_flat[0 : C1_MAX + 1][bass.ds(c1, 1)].offset
            ap1 = bass.AP(out_flat.tensor, d1, [[M + 1, 1]])
            with nc.allow_non_contiguous_dma(reason="single diag cell"):
                sync.dma_start(out=ap1, in_=dtile[4:5, 8:9]).then_inc(fdma_sem, 16)
            # 4x4 corner block, static local position (values 0 off core 7)
            blk_dst = bass.AP(out_flat.tensor, BLK_R0 * M + N, [[M, 4], [1, 4]])
            with nc.allow_non_contiguous_dma(reason="corner block"):
                sync.dma_start(out=blk_dst, in_=dtile[0:4, 8:12]).then_inc(fdma_sem, 16)
            sync.wait_ge(fdma_sem, 64)
    return nc


def _in_maps():
    maps = []
    for r in range(N_CORES):
        g = r * R + np.arange(R)
        vals = np.where(g < N, 1.0, np.where(g < M, 0.25, 0.0)).astype(np.float32)
        dvals = np.zeros((128, 12), np.float32)
        dvals[:, 0:4] = vals[0:H].reshape(128, 4)
        dvals[:, 4:8] = vals[H : 2 * H].reshape(128, 4)
        dvals[4, 8] = vals[2 * H]
        if r == N_CORES - 1:
            dvals[0:4, 8:12] = 0.25
        offs = np.zeros((1, 4), np.int32)
        c0 = r * R
        offs[0, 0] = c0
        # core 7's row-1024 (global 8199) is trimmed garbage; aim its
        # 0.0-valued write at a safe in-bounds spot in a garbage row.
        offs[0, 1] = c0 + 2 * H * (M + 1) if r < N_CORES - 1 else (R - 4) * M
        offs[0, 2] = c0 + H * (M + 1)
        maps.append({"dvals": dvals, "offs": offs})
    return maps


def _run(trace=False, **kwargs):
    global _nc_cache
    if _nc_cache is None:
        _nc_cache = _build()
    return run_bass_kernel_spmd(
        _nc_cache, _in_maps(), core_ids=list(range(N_CORES)), trace=trace, **kwargs
    )


def kernel(x: np.ndarray) -> np.ndarray:
    assert x.shape == (N, 2048), x.shape
    res = _run()
    blocks = [res.results[r]["out"] for r in range(N_CORES)]
    return np.concatenate(blocks, axis=0)[:M]


if __name__ == "__main__":
    out = kernel(np.zeros((N, 2048), np.float32))
    print(out.shape, out.dtype)
